# revision 1
# baseline (speedup 1.0000x reference)
"""Trainium2 Bass kernel for MatrixMPowerSeriesLayer.

Computes res = sum_{i=0}^{7} C_i @ X^i for a batch of 64 complex 512x512
matrices (real/imag stacked), data-parallel over batch across 8 NeuronCores.

Algorithm (per batch element):
  Transposed Horner:  G_7 = C_7^T;  G_k = C_k^T + X^T @ G_{k+1}  (k = 6..0)
  => G_0 = res^T.  On the PE, out = lhsT.T @ rhs, so X^T @ G needs lhsT = X
  (untransposed!) and rhs = G: no transposes on device at all.  Host feeds
  coefficients transposed and transposes the result back.

  Complex multiply via Karatsuba (3 real matmuls instead of 4):
    T1 = Xr^T Gr, T2 = Xi^T Gi, T3 = (Xr+Xi)^T (Gr+Gi)
    real = Cr + T1 - T2, imag = Ci + T3 - T1 - T2
  The C-terms are folded into PSUM with identity-matmul "seeds":
    bank1 = T1
    bank2 = -Cr + T2        (seed rhs = -Cr^T, host-precomputed)
    bank3 = (Ci - Cr) + T3  (seed rhs = (Ci-Cr)^T, host-precomputed)
    Gr_new = bank1 - bank2                (1 DVE op)
    Gi_new = bank3 - bank1 - bank2        (2 DVE ops)
    Gs_new = Gr_new + Gi_new              (1 DVE op, rhs of next step's T3)

  All matmuls run as float32r (FP22): full bf16-rate on the PE at N=512
  with ~11 mantissa bits (measured end-to-end rel err ~5e-4 vs fp32).
"""

import numpy as np
from contextlib import ExitStack

import concourse.bass as bass
from concourse import bacc
import concourse.mybir as mybir
import concourse.tile as tile
from concourse.bass_utils import run_bass_kernel_spmd

B, N, DEG = 64, 512, 8
P = 128
KO = N // P          # 4 partition-chunks per 512 dim
NCORES = 8
BPC = B // NCORES    # 8 batch elements per core
F32 = mybir.dt.float32
F32R = mybir.dt.float32r

_NC_CACHE: dict = {}


def _build_nc(bpc: int = BPC, deg: int = DEG, coeff_bufs: int = 3) -> bass.Bass:
    """Build the per-core Bass program (SPMD; same program on all cores)."""
    nc = bacc.Bacc()

    # DRAM inputs (per core). Layout [*, KO, P, N]: row r of a 512x512 matrix
    # lives at [r // 128, r % 128, :], so each [P, N] chunk is contiguous.
    xr_d = nc.declare_dram_parameter("xr", [bpc, KO, P, N], F32R, isOutput=False)
    xi_d = nc.declare_dram_parameter("xi", [bpc, KO, P, N], F32R, isOutput=False)
    xs_d = nc.declare_dram_parameter("xs", [bpc, KO, P, N], F32R, isOutput=False)
    id_d = nc.declare_dram_parameter("ident", [P, P], F32R, isOutput=False)
    # Seeds for k = deg-2 .. 0 (index j corresponds to k = j)
    ncr_d = nc.declare_dram_parameter("ncr", [deg - 1, KO, P, N], F32R, isOutput=False)
    dct_d = nc.declare_dram_parameter("dct", [deg - 1, KO, P, N], F32R, isOutput=False)
    # Initial state G_{deg-1} = C_{deg-1}^T (shared by all batch elements)
    g0r_d = nc.declare_dram_parameter("g0r", [KO, P, N], F32R, isOutput=False)
    g0i_d = nc.declare_dram_parameter("g0i", [KO, P, N], F32R, isOutput=False)
    g0s_d = nc.declare_dram_parameter("g0s", [KO, P, N], F32R, isOutput=False)

    or_d = nc.declare_dram_parameter("o_r", [bpc, KO, P, N], F32R, isOutput=True)
    oi_d = nc.declare_dram_parameter("o_i", [bpc, KO, P, N], F32R, isOutput=True)

    with tile.TileContext(nc) as tc, ExitStack() as ctx:
        xp = ctx.enter_context(tc.tile_pool(name="xp", bufs=2))
        gp = ctx.enter_context(tc.tile_pool(name="gp", bufs=2))
        cp = ctx.enter_context(tc.tile_pool(name="cp", bufs=coeff_bufs))
        kp = ctx.enter_context(tc.tile_pool(name="kp", bufs=1))
        ps = ctx.enter_context(tc.tile_pool(name="ps", bufs=2, space="PSUM"))

        ident = kp.tile([P, P], F32R, name="ident")
        nc.sync.dma_start(out=ident[:], in_=id_d[:])
        ident_r = ident[:]

        for b in range(bpc):
            # Load this element's X tiles (lhsT operands; partition = k dim)
            xr_t = xp.tile([P, KO, N], F32R, tag="xr", name=f"xr{b}")
            xi_t = xp.tile([P, KO, N], F32R, tag="xi", name=f"xi{b}")
            xs_t = xp.tile([P, KO, N], F32R, tag="xs", name=f"xs{b}")
            for ko in range(KO):
                nc.sync.dma_start(out=xr_t[:, ko, :], in_=xr_d[b, ko])
                nc.sync.dma_start(out=xi_t[:, ko, :], in_=xi_d[b, ko])
                nc.sync.dma_start(out=xs_t[:, ko, :], in_=xs_d[b, ko])

            # Init Horner state G = C_{deg-1}^T
            gr = gp.tile([P, KO, N], F32R, tag="gr", name=f"gr{b}_init")
            gi = gp.tile([P, KO, N], F32R, tag="gi", name=f"gi{b}_init")
            gs = gp.tile([P, KO, N], F32R, tag="gs", name=f"gs{b}_init")
            for ko in range(KO):
                nc.sync.dma_start(out=gr[:, ko, :], in_=g0r_d[ko])
                nc.sync.dma_start(out=gi[:, ko, :], in_=g0i_d[ko])
                nc.sync.dma_start(out=gs[:, ko, :], in_=g0s_d[ko])

            for k in range(deg - 2, -1, -1):
                last = k == 0
                ncr_t = cp.tile([P, KO, N], F32R, tag="ncr", name=f"ncr{b}_{k}")
                dct_t = cp.tile([P, KO, N], F32R, tag="dct", name=f"dct{b}_{k}")
                for ko in range(KO):
                    nc.sync.dma_start(out=ncr_t[:, ko, :], in_=ncr_d[k, ko])
                    nc.sync.dma_start(out=dct_t[:, ko, :], in_=dct_d[k, ko])

                gr_n = gp.tile([P, KO, N], F32R, tag="gr", name=f"gr{b}_{k}")
                gi_n = gp.tile([P, KO, N], F32R, tag="gi", name=f"gi{b}_{k}")
                gs_n = (
                    None
                    if last
                    else gp.tile([P, KO, N], F32R, tag="gs", name=f"gs{b}_{k}")
                )

                for m in range(KO):
                    msl = slice(m * P, (m + 1) * P)
                    t1 = ps.tile([P, N], F32, tag="t1", name=f"t1_{b}_{k}_{m}")
                    t2 = ps.tile([P, N], F32, tag="t2", name=f"t2_{b}_{k}_{m}")
                    t3 = ps.tile([P, N], F32, tag="t3", name=f"t3_{b}_{k}_{m}")

                    # bank1 = T1 = (Xr^T Gr)[m]
                    for ko in range(KO):
                        nc.tensor.matmul(
                            t1[:],
                            lhsT=xr_t[:, ko, msl],
                            rhs=gr[:, ko, :],
                            start=(ko == 0),
                            stop=(ko == KO - 1),
                        )
                    # bank2 = -Cr^T[m] + T2[m]
                    nc.tensor.matmul(
                        t2[:],
                        lhsT=ident_r,
                        rhs=ncr_t[:, m, :],
                        start=True,
                        stop=False,
                    )
                    for ko in range(KO):
                        nc.tensor.matmul(
                            t2[:],
                            lhsT=xi_t[:, ko, msl],
                            rhs=gi[:, ko, :],
                            start=False,
                            stop=(ko == KO - 1),
                        )
                    # bank3 = (Ci-Cr)^T[m] + T3[m]
                    nc.tensor.matmul(
                        t3[:],
                        lhsT=ident_r,
                        rhs=dct_t[:, m, :],
                        start=True,
                        stop=False,
                    )
                    for ko in range(KO):
                        nc.tensor.matmul(
                            t3[:],
                            lhsT=xs_t[:, ko, msl],
                            rhs=gs[:, ko, :],
                            start=False,
                            stop=(ko == KO - 1),
                        )

                    # DVE may read only ONE PSUM operand per op: stage T1 to
                    # SBUF on ScalarE, then chain single-PSUM DVE ops.
                    u = kp.tile([P, N], F32, tag="u", bufs=3, name=f"u_{b}_{k}_{m}")
                    nc.scalar.copy(u[:], t1[:])
                    # Gr_new[m] = T1 - bank2 = Cr + T1 - T2
                    nc.vector.tensor_sub(gr_n[:, m, :], u[:], t2[:])
                    # Gi_new[m] = bank3 - T1 - bank2 = Ci + T3 - T1 - T2
                    nc.vector.tensor_sub(gi_n[:, m, :], t3[:], u[:])
                    nc.vector.tensor_sub(gi_n[:, m, :], gi_n[:, m, :], t2[:])
                    if last:
                        nc.sync.dma_start(out=or_d[b, m], in_=gr_n[:, m, :])
                        nc.sync.dma_start(out=oi_d[b, m], in_=gi_n[:, m, :])
                    else:
                        nc.vector.tensor_add(gs_n[:, m, :], gr_n[:, m, :], gi_n[:, m, :])

                gr, gi, gs = gr_n, gi_n, gs_n

    nc.finalize()
    return nc


def _get_nc() -> bass.Bass:
    if "nc" not in _NC_CACHE:
        _NC_CACHE["nc"] = _build_nc()
    return _NC_CACHE["nc"]


def _prep_inputs(x: np.ndarray, coeffs: np.ndarray):
    """Host-side prep: tile/transpose into the DRAM layouts the kernel wants."""
    x = np.ascontiguousarray(x, dtype=np.float32)
    coeffs = np.ascontiguousarray(coeffs, dtype=np.float32)

    xr = x[:, 0].reshape(B, KO, P, N)
    xi = x[:, 1].reshape(B, KO, P, N)
    xs = (x[:, 0] + x[:, 1]).reshape(B, KO, P, N)

    crT = np.ascontiguousarray(coeffs[:, 0].transpose(0, 2, 1))  # [DEG, N, N]
    ciT = np.ascontiguousarray(coeffs[:, 1].transpose(0, 2, 1))
    ncr = np.ascontiguousarray(-crT[: DEG - 1]).reshape(DEG - 1, KO, P, N)
    dct = np.ascontiguousarray(ciT[: DEG - 1] - crT[: DEG - 1]).reshape(
        DEG - 1, KO, P, N
    )
    g0r = np.ascontiguousarray(crT[DEG - 1]).reshape(KO, P, N)
    g0i = np.ascontiguousarray(ciT[DEG - 1]).reshape(KO, P, N)
    g0s = np.ascontiguousarray(crT[DEG - 1] + ciT[DEG - 1]).reshape(KO, P, N)

    ident = np.eye(P, dtype=np.float32)

    in_maps = []
    for c in range(NCORES):
        sl = slice(c * BPC, (c + 1) * BPC)
        in_maps.append(
            {
                "ident": ident,
                "xr": np.ascontiguousarray(xr[sl]),
                "xi": np.ascontiguousarray(xi[sl]),
                "xs": np.ascontiguousarray(xs[sl]),
                "ncr": ncr,
                "dct": dct,
                "g0r": g0r,
                "g0i": g0i,
                "g0s": g0s,
            }
        )
    return in_maps


def _assemble_output(results) -> np.ndarray:
    out = np.empty((B, 2, N, N), dtype=np.float32)
    for c in range(NCORES):
        o_r = results[c]["o_r"].reshape(BPC, N, N)
        o_i = results[c]["o_i"].reshape(BPC, N, N)
        for b in range(BPC):
            out[c * BPC + b, 0] = o_r[b].T
            out[c * BPC + b, 1] = o_i[b].T
    return out


def run_sharded(x: np.ndarray, coeffs: np.ndarray, **run_kwargs):
    """Run the SPMD kernel on 8 cores; returns (output, BassKernelResults)."""
    nc = _get_nc()
    in_maps = _prep_inputs(x, coeffs)
    res = run_bass_kernel_spmd(nc, in_maps, list(range(NCORES)), **run_kwargs)
    return _assemble_output(res.results), res


def kernel(x: np.ndarray, coeffs: np.ndarray) -> np.ndarray:
    out, _ = run_sharded(x, coeffs)
    return out



# revision 3
# speedup vs baseline: 1.2255x; 1.2255x over previous
"""Trainium2 Bass kernel v3 for MatrixMPowerSeriesLayer.

res = sum_{i=0}^{7} C_i @ X^i, batch 64 complex 512x512, data-parallel over
batch on 8 NeuronCores (8 elements/core).

Transposed Horner:  G_7 = C_7^T;  G_k = C_k^T + X^T @ G_{k+1};  G_0 = res^T.
PE computes X^T @ G with lhsT = X (untransposed), rhs = G.

Hybrid precision/algorithm (error budget: harness gate is rel_l2 < 2e-2,
measured here ~6e-3):
  - Steps k = 6..3 (early steps; their error is damped by ~0.45^k through
    the remaining X-multiplies): fp8e4 DoubleRow schoolbook complex multiply.
    DR runs contraction 256 per MM at the same 216ns cadence as a regular
    MM -> 2x throughput. Schoolbook (4 products, with -Xi precomputed) needs
    no Gr+Gi running sum and only one DVE add per produced component:
      T_i = Xr^T Gi + Xi^T Gr      (4 DR MMs, one PSUM bank)
      T_r = Xr^T Gr + (-Xi)^T Gi   (4 DR MMs)
      Gi' = Ci^T + T_i ; Gr' = Cr^T + T_r
  - Steps k = 2..0: bf16 Karatsuba (3 products = 12 MMs/chunk):
      T1 = Xr^T Gr, T2 = Xi^T Gi, T3 = (Xr+Xi)^T (Gr+Gi), PSUM banks
      ordered (T3, T1, T2) so one wide ScalarE ACTIVATE stages (w,u,v) and
      the DVE chain is two fused [P,2,N] bf16 2x-mode ops + two singles:
        e2 = (w-u, u-v) ; e2[0] -= v  -> (c1, a)
        g2 = e2 + (Ci^T, Cr^T)       -> (Gi', Gr')
        gs = Gi' + Gr'
  - State layout g2 = [P, KO, 2, N] with j=0 -> Gi, j=1 -> Gr; a DR MM
    reads rhs pairs g2[:, 2c:2c+2, j, :].
  - Coefficients resident in SBUF (packed (Ci,Cr) interleaved), G_7 = C_7^T
    read directly from resident tiles, outputs written bf16 and upcast on
    host (output quantization is not compounded).
"""

import numpy as np
import ml_dtypes
from contextlib import ExitStack

import concourse.bass as bass
from concourse import bacc
import concourse.mybir as mybir
import concourse.tile as tile
from concourse.bass_utils import run_bass_kernel_spmd

B, N, DEG = 64, 512, 8
P = 128
KO = N // P
NCORES = 8
BPC = B // NCORES
F32 = mybir.dt.float32
BF16 = mybir.dt.bfloat16
FP8 = mybir.dt.float8e4
BF16_NP = ml_dtypes.bfloat16
FP8_NP = ml_dtypes.float8_e4m3

N_FP8_STEPS = 4          # steps k = DEG-2 .. DEG-1-N_FP8_STEPS run in fp8 DR
DR = mybir.MatmulPerfMode.DoubleRow

_NC_CACHE: dict = {}


def _build_nc(bpc: int = BPC, deg: int = DEG) -> bass.Bass:
    nc = bacc.Bacc()
    fp8_min_k = deg - 1 - N_FP8_STEPS    # k >= fp8_min_k -> fp8 step

    xr_d = nc.declare_dram_parameter("xr", [bpc, KO, P, N], BF16, isOutput=False)
    xi_d = nc.declare_dram_parameter("xi", [bpc, KO, P, N], BF16, isOutput=False)
    xs_d = nc.declare_dram_parameter("xs", [bpc, KO, P, N], BF16, isOutput=False)
    xr8_d = nc.declare_dram_parameter("xr8", [bpc, KO, P, N], FP8, isOutput=False)
    xi8_d = nc.declare_dram_parameter("xi8", [bpc, KO, P, N], FP8, isOutput=False)
    xn8_d = nc.declare_dram_parameter("xn8", [bpc, KO, P, N], FP8, isOutput=False)
    # packed coeffs: cc[k, ko, j, :, :] with j=0 -> Ci^T, j=1 -> Cr^T (bf16)
    cc_d = nc.declare_dram_parameter("cc", [deg - 1, KO, 2, P, N], BF16, isOutput=False)
    # cs = (Cr+Ci)^T for the fp8->bf16 transition step k = fp8_min_k+1
    cst_d = nc.declare_dram_parameter("cst", [KO, P, N], BF16, isOutput=False)
    # G init (C7^T), fp8, packed (Gi, Gr)
    g0c8_d = nc.declare_dram_parameter("g0c8", [KO, 2, P, N], FP8, isOutput=False)

    # output: packed (imag, real) bf16, host up-casts + transposes
    oc_d = nc.declare_dram_parameter("oc", [bpc, KO, P, 2, N], BF16, isOutput=True)

    with tile.TileContext(nc) as tc, ExitStack() as ctx:
        cp = ctx.enter_context(tc.tile_pool(name="cp", bufs=1))
        xp = ctx.enter_context(tc.tile_pool(name="xp", bufs=2))
        gp = ctx.enter_context(tc.tile_pool(name="gp", bufs=2))
        up = ctx.enter_context(tc.tile_pool(name="up", bufs=4))
        ps = ctx.enter_context(tc.tile_pool(name="ps", bufs=2, space="PSUM"))

        cc = cp.tile([P, deg - 1, KO, 2, N], BF16, name="cc")
        cst = cp.tile([P, KO, N], BF16, name="cst")
        g0c8 = cp.tile([P, KO, 2, N], FP8, name="g0c8")

        # --- DMA emission in need-order ---
        for ko in range(KO):
            for j in range(2):
                nc.sync.dma_start(out=g0c8[:, ko, j, :], in_=g0c8_d[ko, j])
        x_tiles = {}

        def load_x(b):
            t = {}
            for nm, dd, dt in (
                ("xr8", xr8_d, FP8), ("xi8", xi8_d, FP8), ("xn8", xn8_d, FP8),
                ("xr", xr_d, BF16), ("xi", xi_d, BF16), ("xs", xs_d, BF16),
            ):
                tt = xp.tile([P, KO, N], dt, tag=nm, name=f"{nm}{b}")
                for ko in range(KO):
                    nc.sync.dma_start(out=tt[:, ko, :], in_=dd[b, ko])
                t[nm] = tt
            x_tiles[b] = t
        load_x(0)
        for k in range(deg - 2, -1, -1):
            for ko in range(KO):
                for j in range(2):
                    nc.sync.dma_start(out=cc[:, k, ko, j, :], in_=cc_d[k, ko, j])
        for ko in range(KO):
            nc.sync.dma_start(out=cst[:, ko, :], in_=cst_d[ko])

        for b in range(bpc):
            if b not in x_tiles:
                load_x(b)
            xt = x_tiles.pop(b)
            xr8, xi8, xn8 = xt["xr8"], xt["xi8"], xt["xn8"]
            xr, xi, xs = xt["xr"], xt["xi"], xt["xs"]

            g2 = None          # [P, KO, 2, N] (Gi, Gr), fp8 or bf16
            gs = None          # [P, KO, N] bf16 (Karatsuba steps only)
            for k in range(deg - 2, -1, -1):
                fp8_step = k >= fp8_min_k
                trans = k == fp8_min_k       # last fp8 step: emit bf16 + gs
                last = k == 0

                if fp8_step:
                    out_dt = BF16 if trans else FP8
                    out_tag = "g2b" if trans else "g28"
                    g2_n = gp.tile([P, KO, 2, N], out_dt, tag=out_tag,
                                   name=f"g2_{b}_{k}")
                    gs_n = (
                        gp.tile([P, KO, N], BF16, tag="gs", name=f"gs{b}_{k}")
                        if trans else None
                    )
                    for m in range(KO):
                        msl = slice(m * P, (m + 1) * P)
                        t = ps.tile([P, 3, N], F32, tag="t", name=f"t_{b}_{k}_{m}")
                        # T_i -> slice 0, T_r -> slice 1 (slice 2 unused).
                        # T_r first so its staging copy + C-add can run while
                        # T_i's MMs stream.
                        for c in range(KO // 2):
                            pr = slice(2 * c, 2 * c + 2)
                            rhs_r = (g0c8 if g2 is None else g2)[:, pr, 1, :]
                            nc.tensor.matmul(
                                t[:, 1, :], lhsT=xr8[:, pr, msl], rhs=rhs_r,
                                start=(c == 0), stop=False, perf_mode=DR,
                            )
                        for c in range(KO // 2):
                            pr = slice(2 * c, 2 * c + 2)
                            rhs_i = (g0c8 if g2 is None else g2)[:, pr, 0, :]
                            nc.tensor.matmul(
                                t[:, 1, :], lhsT=xn8[:, pr, msl], rhs=rhs_i,
                                start=False, stop=(c == KO // 2 - 1),
                                perf_mode=DR,
                            )
                        for c in range(KO // 2):
                            pr = slice(2 * c, 2 * c + 2)
                            rhs_i = (g0c8 if g2 is None else g2)[:, pr, 0, :]
                            nc.tensor.matmul(
                                t[:, 0, :], lhsT=xr8[:, pr, msl], rhs=rhs_i,
                                start=(c == 0), stop=False, perf_mode=DR,
                            )
                        for c in range(KO // 2):
                            pr = slice(2 * c, 2 * c + 2)
                            rhs_r = (g0c8 if g2 is None else g2)[:, pr, 1, :]
                            nc.tensor.matmul(
                                t[:, 0, :], lhsT=xi8[:, pr, msl], rhs=rhs_r,
                                start=False, stop=(c == KO // 2 - 1),
                                perf_mode=DR,
                            )

                        # G' = (T_psum * unscale) + C directly on the DVE --
                        # no ScalarE staging at all in fp8 steps.  X is
                        # shipped as fp8(64*X), G stored as fp8(16*G), so a
                        # product in PSUM carries 2^10; unscale keeps the
                        # next G at 16x (fp8 steps) or 1x (transition).
                        unscale = 2.0 ** -10 if trans else 2.0 ** -6
                        nc.vector.scalar_tensor_tensor(
                            g2_n[:, m, 1, :], t[:, 1, :], unscale,
                            cc[:, k, m, 1, :],
                            op0=mybir.AluOpType.mult, op1=mybir.AluOpType.add,
                        )
                        nc.vector.scalar_tensor_tensor(
                            g2_n[:, m, 0, :], t[:, 0, :], unscale,
                            cc[:, k, m, 0, :],
                            op0=mybir.AluOpType.mult, op1=mybir.AluOpType.add,
                        )
                        if trans:
                            nc.vector.tensor_add(
                                gs_n[:, m, :], g2_n[:, m, 0, :], g2_n[:, m, 1, :]
                            )
                    g2, gs = g2_n, gs_n

                else:
                    # bf16 Karatsuba step; PSUM order (T3, T1, T2)
                    if not last:
                        g2_n = gp.tile([P, KO, 2, N], BF16, tag="g2b",
                                       name=f"g2_{b}_{k}")
                        gs_n = gp.tile([P, KO, N], BF16, tag="gs",
                                       name=f"gs{b}_{k}")
                    for m in range(KO):
                        msl = slice(m * P, (m + 1) * P)
                        t = ps.tile([P, 3, N], F32, tag="t", name=f"t_{b}_{k}_{m}")
                        for ko in range(KO):   # T1 = Xr Gr -> slice 1
                            nc.tensor.matmul(
                                t[:, 1, :], lhsT=xr[:, ko, msl],
                                rhs=g2[:, ko, 1, :],
                                start=(ko == 0), stop=(ko == KO - 1),
                            )
                        for ko in range(KO):   # T2 = Xi Gi -> slice 2
                            nc.tensor.matmul(
                                t[:, 2, :], lhsT=xi[:, ko, msl],
                                rhs=g2[:, ko, 0, :],
                                start=(ko == 0), stop=(ko == KO - 1),
                            )
                        for ko in range(KO):   # T3 = Xs Gs -> slice 0
                            nc.tensor.matmul(
                                t[:, 0, :], lhsT=xs[:, ko, msl],
                                rhs=gs[:, ko, :],
                                start=(ko == 0), stop=(ko == KO - 1),
                            )

                        uvw = up.tile([P, 3, N], BF16, tag="uvw",
                                      name=f"uvw_{b}_{k}_{m}")
                        if m == KO - 1:
                            # split: (T3,T1) after T1 stops, T2 after T2 stops
                            nc.scalar.copy(uvw[:, 0:2, :], t[:, 0:2, :])
                            nc.scalar.copy(uvw[:, 2, :], t[:, 2, :])
                        else:
                            nc.scalar.copy(uvw[:], t[:])
                        w, u, v = uvw[:, 0, :], uvw[:, 1, :], uvw[:, 2, :]

                        # e2 = (w-u, u-v) = (c0, a); then c0 -= v -> c1
                        e2 = up.tile([P, 2, N], BF16, tag="e2",
                                     name=f"e2_{b}_{k}_{m}")
                        nc.vector.tensor_sub(e2[:], uvw[:, 0:2, :], uvw[:, 1:3, :])
                        nc.vector.tensor_sub(e2[:, 0, :], e2[:, 0, :], v)
                        if last:
                            f2 = up.tile([P, 2, N], BF16, tag="f2",
                                         name=f"f2_{b}_{m}")
                            nc.vector.tensor_add(f2[:], e2[:], cc[:, k, m, :, :])
                            nc.sync.dma_start(out=oc_d[b, m], in_=f2[:])
                        else:
                            nc.vector.tensor_add(
                                g2_n[:, m, :, :], e2[:], cc[:, k, m, :, :]
                            )
                            nc.vector.tensor_add(
                                gs_n[:, m, :], g2_n[:, m, 0, :], g2_n[:, m, 1, :]
                            )
                    if not last:
                        g2, gs = g2_n, gs_n

            if b + 1 < bpc:
                load_x(b + 1)

    nc.finalize()
    return nc


def _get_nc() -> bass.Bass:
    if "nc" not in _NC_CACHE:
        _NC_CACHE["nc"] = _build_nc()
    return _NC_CACHE["nc"]


def _prep_inputs(x: np.ndarray, coeffs: np.ndarray):
    x = np.ascontiguousarray(x, dtype=np.float32)
    coeffs = np.ascontiguousarray(coeffs, dtype=np.float32)

    xr_f = x[:, 0].reshape(B, KO, P, N)
    xi_f = x[:, 1].reshape(B, KO, P, N)
    xr = xr_f.astype(BF16_NP)
    xi = xi_f.astype(BF16_NP)
    xs = (xr_f + xi_f).astype(BF16_NP)
    # fp8 operands are pre-scaled into e4m3's normal range (X entries are
    # ~N(0, 0.02^2) -- raw they'd be almost entirely subnormal):
    #   X shipped as fp8(64*X), G kept as fp8(16*G).
    xr8 = (xr_f * 64).astype(FP8_NP)
    xi8 = (xi_f * 64).astype(FP8_NP)
    xn8 = (-xi_f * 64).astype(FP8_NP)

    crT = np.ascontiguousarray(coeffs[:, 0].transpose(0, 2, 1))  # [DEG, N, N]
    ciT = np.ascontiguousarray(coeffs[:, 1].transpose(0, 2, 1))
    # cc[k, ko, j, p, n]: j=0 -> Ci^T, j=1 -> Cr^T.  For the non-transition
    # fp8 steps (k > k_trans) the coefficients carry the 16x G-scale.
    k_trans = DEG - 1 - N_FP8_STEPS
    cc_f = np.stack(
        [ciT[: DEG - 1].reshape(DEG - 1, KO, P, N),
         crT[: DEG - 1].reshape(DEG - 1, KO, P, N)], axis=2
    ).copy()
    cc_f[k_trans + 1 :] *= 16.0
    cc = cc_f.astype(BF16_NP)
    cst = (crT[k_trans] + ciT[k_trans]).reshape(KO, P, N).astype(BF16_NP)
    g0c8 = (16.0 * np.stack(
        [ciT[DEG - 1].reshape(KO, P, N), crT[DEG - 1].reshape(KO, P, N)], axis=1
    )).astype(FP8_NP)

    in_maps = []
    for c in range(NCORES):
        sl = slice(c * BPC, (c + 1) * BPC)
        in_maps.append(
            {
                "xr": np.ascontiguousarray(xr[sl]),
                "xi": np.ascontiguousarray(xi[sl]),
                "xs": np.ascontiguousarray(xs[sl]),
                "xr8": np.ascontiguousarray(xr8[sl]),
                "xi8": np.ascontiguousarray(xi8[sl]),
                "xn8": np.ascontiguousarray(xn8[sl]),
                "cc": cc,
                "cst": cst,
                "g0c8": g0c8,
            }
        )
    return in_maps


def _assemble_output(results) -> np.ndarray:
    out = np.empty((B, 2, N, N), dtype=np.float32)
    for c in range(NCORES):
        oc = results[c]["oc"].astype(np.float32)      # [BPC, KO, P, 2, N]
        re = oc[:, :, :, 1, :].reshape(BPC, N, N)     # res^T rows = ko*P+p
        im = oc[:, :, :, 0, :].reshape(BPC, N, N)
        for b in range(BPC):
            out[c * BPC + b, 0] = re[b].T
            out[c * BPC + b, 1] = im[b].T
    return out


def run_sharded(x: np.ndarray, coeffs: np.ndarray, **run_kwargs):
    nc = _get_nc()
    in_maps = _prep_inputs(x, coeffs)
    res = run_bass_kernel_spmd(nc, in_maps, list(range(NCORES)), **run_kwargs)
    return _assemble_output(res.results), res


def kernel(x: np.ndarray, coeffs: np.ndarray) -> np.ndarray:
    out, _ = run_sharded(x, coeffs)
    return out


# revision 4
# speedup vs baseline: 1.2276x; 1.0017x over previous
"""Trainium2 Bass kernel v3 for MatrixMPowerSeriesLayer.

res = sum_{i=0}^{7} C_i @ X^i, batch 64 complex 512x512, data-parallel over
batch on 8 NeuronCores (8 elements/core).

Transposed Horner:  G_7 = C_7^T;  G_k = C_k^T + X^T @ G_{k+1};  G_0 = res^T.
PE computes X^T @ G with lhsT = X (untransposed), rhs = G.

Hybrid precision/algorithm (error budget: harness gate is rel_l2 < 2e-2,
measured here ~6e-3):
  - Steps k = 6..3 (early steps; their error is damped by ~0.45^k through
    the remaining X-multiplies): fp8e4 DoubleRow schoolbook complex multiply.
    DR runs contraction 256 per MM at the same 216ns cadence as a regular
    MM -> 2x throughput. Schoolbook (4 products, with -Xi precomputed) needs
    no Gr+Gi running sum and only one DVE add per produced component:
      T_i = Xr^T Gi + Xi^T Gr      (4 DR MMs, one PSUM bank)
      T_r = Xr^T Gr + (-Xi)^T Gi   (4 DR MMs)
      Gi' = Ci^T + T_i ; Gr' = Cr^T + T_r
  - Steps k = 2..0: bf16 Karatsuba (3 products = 12 MMs/chunk):
      T1 = Xr^T Gr, T2 = Xi^T Gi, T3 = (Xr+Xi)^T (Gr+Gi), PSUM banks
      ordered (T3, T1, T2) so one wide ScalarE ACTIVATE stages (w,u,v) and
      the DVE chain is two fused [P,2,N] bf16 2x-mode ops + two singles:
        e2 = (w-u, u-v) ; e2[0] -= v  -> (c1, a)
        g2 = e2 + (Ci^T, Cr^T)       -> (Gi', Gr')
        gs = Gi' + Gr'
  - State layout g2 = [P, KO, 2, N] with j=0 -> Gi, j=1 -> Gr; a DR MM
    reads rhs pairs g2[:, 2c:2c+2, j, :].
  - Coefficients resident in SBUF (packed (Ci,Cr) interleaved), G_7 = C_7^T
    read directly from resident tiles, outputs written bf16 and upcast on
    host (output quantization is not compounded).
"""

import numpy as np
import ml_dtypes
from contextlib import ExitStack

import concourse.bass as bass
from concourse import bacc
import concourse.mybir as mybir
import concourse.tile as tile
from concourse.bass_utils import run_bass_kernel_spmd

B, N, DEG = 64, 512, 8
P = 128
KO = N // P
NCORES = 8
BPC = B // NCORES
F32 = mybir.dt.float32
BF16 = mybir.dt.bfloat16
FP8 = mybir.dt.float8e4
BF16_NP = ml_dtypes.bfloat16
FP8_NP = ml_dtypes.float8_e4m3

N_FP8_STEPS = 4          # steps k = DEG-2 .. DEG-1-N_FP8_STEPS run in fp8 DR
DR = mybir.MatmulPerfMode.DoubleRow

_NC_CACHE: dict = {}


def _build_nc(bpc: int = BPC, deg: int = DEG) -> bass.Bass:
    nc = bacc.Bacc()
    fp8_min_k = deg - 1 - N_FP8_STEPS    # k >= fp8_min_k -> fp8 step

    xr_d = nc.declare_dram_parameter("xr", [bpc, KO, P, N], BF16, isOutput=False)
    xi_d = nc.declare_dram_parameter("xi", [bpc, KO, P, N], BF16, isOutput=False)
    xs_d = nc.declare_dram_parameter("xs", [bpc, KO, P, N], BF16, isOutput=False)
    xr8_d = nc.declare_dram_parameter("xr8", [bpc, KO, P, N], FP8, isOutput=False)
    xi8_d = nc.declare_dram_parameter("xi8", [bpc, KO, P, N], FP8, isOutput=False)
    xn8_d = nc.declare_dram_parameter("xn8", [bpc, KO, P, N], FP8, isOutput=False)
    # packed coeffs: cc[k, ko, j, :, :] with j=0 -> Ci^T, j=1 -> Cr^T (bf16)
    cc_d = nc.declare_dram_parameter("cc", [deg - 1, KO, 2, P, N], BF16, isOutput=False)
    # cs = (Cr+Ci)^T for the fp8->bf16 transition step k = fp8_min_k+1
    cst_d = nc.declare_dram_parameter("cst", [KO, P, N], BF16, isOutput=False)
    # G init (C7^T), fp8, packed (Gi, Gr)
    g0c8_d = nc.declare_dram_parameter("g0c8", [KO, 2, P, N], FP8, isOutput=False)

    # output: packed (imag, real) bf16, host up-casts + transposes
    oc_d = nc.declare_dram_parameter("oc", [bpc, KO, P, 2, N], BF16, isOutput=True)

    with tile.TileContext(nc) as tc, ExitStack() as ctx:
        cp = ctx.enter_context(tc.tile_pool(name="cp", bufs=1))
        xp = ctx.enter_context(tc.tile_pool(name="xp", bufs=2))
        gp = ctx.enter_context(tc.tile_pool(name="gp", bufs=2))
        up = ctx.enter_context(tc.tile_pool(name="up", bufs=6))
        ps = ctx.enter_context(tc.tile_pool(name="ps", bufs=2, space="PSUM"))

        cc = cp.tile([P, deg - 1, KO, 2, N], BF16, name="cc")
        cst = cp.tile([P, KO, N], BF16, name="cst")
        g0c8 = cp.tile([P, KO, 2, N], FP8, name="g0c8")

        # --- DMA emission in need-order ---
        for ko in range(KO):
            for j in range(2):
                nc.sync.dma_start(out=g0c8[:, ko, j, :], in_=g0c8_d[ko, j])
        x_tiles = {}

        def load_x(b, kinds):
            t = x_tiles.setdefault(b, {})
            specs = {
                "xr8": (xr8_d, FP8), "xi8": (xi8_d, FP8), "xn8": (xn8_d, FP8),
                "xr": (xr_d, BF16), "xi": (xi_d, BF16), "xs": (xs_d, BF16),
            }
            for nm in kinds:
                dd, dt = specs[nm]
                tt = xp.tile([P, KO, N], dt, tag=nm, name=f"{nm}{b}")
                for ko in range(KO):
                    nc.sync.dma_start(out=tt[:, ko, :], in_=dd[b, ko])
                t[nm] = tt

        FP8_X = ("xr8", "xi8", "xn8")
        BF16_X = ("xr", "xi", "xs")
        # b0's fp8 X (needed first) + first-step coeffs before everything else
        load_x(0, FP8_X)
        for ko in range(KO):
            for j in range(2):
                nc.sync.dma_start(out=cc[:, deg - 2, ko, j, :], in_=cc_d[deg - 2, ko, j])
        load_x(0, BF16_X)
        for k in range(deg - 3, -1, -1):
            for ko in range(KO):
                for j in range(2):
                    nc.sync.dma_start(out=cc[:, k, ko, j, :], in_=cc_d[k, ko, j])
        for ko in range(KO):
            nc.sync.dma_start(out=cst[:, ko, :], in_=cst_d[ko])

        for b in range(bpc):
            if b not in x_tiles:
                load_x(b, FP8_X + BF16_X)
            xt = x_tiles.pop(b)
            xr8, xi8, xn8 = xt["xr8"], xt["xi8"], xt["xn8"]
            xr, xi, xs = xt["xr"], xt["xi"], xt["xs"]

            g2 = None          # [P, KO, 2, N] (Gi, Gr), fp8 or bf16
            gs = None          # [P, KO, N] bf16 (Karatsuba steps only)
            for k in range(deg - 2, -1, -1):
                fp8_step = k >= fp8_min_k
                trans = k == fp8_min_k       # last fp8 step: emit bf16 + gs
                last = k == 0

                if fp8_step:
                    out_dt = BF16 if trans else FP8
                    out_tag = "g2b" if trans else "g28"
                    g2_n = gp.tile([P, KO, 2, N], out_dt, tag=out_tag,
                                   name=f"g2_{b}_{k}")
                    gs_n = (
                        gp.tile([P, KO, N], BF16, tag="gs", name=f"gs{b}_{k}")
                        if trans else None
                    )
                    for m in range(KO):
                        msl = slice(m * P, (m + 1) * P)
                        # odd chunks rotate onto a separate 2-bank tag so a
                        # new accumulation never waits on the stt reads of
                        # the chunk 2 back (PSUM WAR).
                        if m % 2 == 0:
                            t = ps.tile([P, 3, N], F32, tag="t", name=f"t_{b}_{k}_{m}")
                        else:
                            t = ps.tile([P, 2, N], F32, tag="t2", bufs=1, name=f"t_{b}_{k}_{m}")
                        # T_i -> slice 0, T_r -> slice 1.
                        # T_r first so its staging copy + C-add can run while
                        # T_i's MMs stream.
                        for c in range(KO // 2):
                            pr = slice(2 * c, 2 * c + 2)
                            rhs_r = (g0c8 if g2 is None else g2)[:, pr, 1, :]
                            nc.tensor.matmul(
                                t[:, 1, :], lhsT=xr8[:, pr, msl], rhs=rhs_r,
                                start=(c == 0), stop=False, perf_mode=DR,
                            )
                        for c in range(KO // 2):
                            pr = slice(2 * c, 2 * c + 2)
                            rhs_i = (g0c8 if g2 is None else g2)[:, pr, 0, :]
                            nc.tensor.matmul(
                                t[:, 1, :], lhsT=xn8[:, pr, msl], rhs=rhs_i,
                                start=False, stop=(c == KO // 2 - 1),
                                perf_mode=DR,
                            )
                        for c in range(KO // 2):
                            pr = slice(2 * c, 2 * c + 2)
                            rhs_i = (g0c8 if g2 is None else g2)[:, pr, 0, :]
                            nc.tensor.matmul(
                                t[:, 0, :], lhsT=xr8[:, pr, msl], rhs=rhs_i,
                                start=(c == 0), stop=False, perf_mode=DR,
                            )
                        for c in range(KO // 2):
                            pr = slice(2 * c, 2 * c + 2)
                            rhs_r = (g0c8 if g2 is None else g2)[:, pr, 1, :]
                            nc.tensor.matmul(
                                t[:, 0, :], lhsT=xi8[:, pr, msl], rhs=rhs_r,
                                start=False, stop=(c == KO // 2 - 1),
                                perf_mode=DR,
                            )

                        # G' = (T_psum * unscale) + C directly on the DVE --
                        # no ScalarE staging at all in fp8 steps.  X is
                        # shipped as fp8(64*X), G stored as fp8(16*G), so a
                        # product in PSUM carries 2^10; unscale keeps the
                        # next G at 16x (fp8 steps) or 1x (transition).
                        unscale = 2.0 ** -10 if trans else 2.0 ** -6
                        nc.vector.scalar_tensor_tensor(
                            g2_n[:, m, 1, :], t[:, 1, :], unscale,
                            cc[:, k, m, 1, :],
                            op0=mybir.AluOpType.mult, op1=mybir.AluOpType.add,
                        )
                        nc.vector.scalar_tensor_tensor(
                            g2_n[:, m, 0, :], t[:, 0, :], unscale,
                            cc[:, k, m, 0, :],
                            op0=mybir.AluOpType.mult, op1=mybir.AluOpType.add,
                        )
                        if trans:
                            nc.vector.tensor_add(
                                gs_n[:, m, :], g2_n[:, m, 0, :], g2_n[:, m, 1, :]
                            )
                    g2, gs = g2_n, gs_n

                else:
                    # bf16 Karatsuba step; PSUM order (T3, T1, T2)
                    if not last:
                        g2_n = gp.tile([P, KO, 2, N], BF16, tag="g2b",
                                       name=f"g2_{b}_{k}")
                        gs_n = gp.tile([P, KO, N], BF16, tag="gs",
                                       name=f"gs{b}_{k}")
                    for m in range(KO):
                        msl = slice(m * P, (m + 1) * P)
                        t = ps.tile([P, 3, N], F32, tag="t", name=f"t_{b}_{k}_{m}")
                        # products: T1 = Xr Gr -> slice 1, T2 = Xi Gi -> 2,
                        # T3 = Xs Gs -> slice 0.  For the first chunk of a
                        # step, push every product's ko=3 MM to the end: the
                        # ko=3 G-slices are the previous step's last DVE
                        # outputs and arrive latest.
                        prods = [
                            (1, xr, lambda ko: g2[:, ko, 1, :]),
                            (2, xi, lambda ko: g2[:, ko, 0, :]),
                            (0, xs, lambda ko: gs[:, ko, :]),
                        ]
                        if m == 0:
                            order = [(s, ko) for s, _, _ in prods for ko in range(KO - 1)]
                            order += [(s, KO - 1) for s, _, _ in prods]
                        else:
                            order = [(s, ko) for s, _, _ in prods for ko in range(KO)]
                        pmap = {s: (xt_, rhs_) for s, xt_, rhs_ in prods}
                        for s, ko in order:
                            xt_, rhs_ = pmap[s]
                            nc.tensor.matmul(
                                t[:, s, :], lhsT=xt_[:, ko, msl],
                                rhs=rhs_(ko),
                                start=(ko == 0), stop=(ko == KO - 1),
                            )

                        uvw = up.tile([P, 3, N], BF16, tag="uvw",
                                      name=f"uvw_{b}_{k}_{m}")
                        if m == KO - 1:
                            # split: (T3,T1) after T1 stops, T2 after T2 stops
                            nc.scalar.copy(uvw[:, 0:2, :], t[:, 0:2, :])
                            nc.scalar.copy(uvw[:, 2, :], t[:, 2, :])
                        else:
                            nc.scalar.copy(uvw[:], t[:])
                        w, u, v = uvw[:, 0, :], uvw[:, 1, :], uvw[:, 2, :]

                        # e2 = (w-u, u-v) = (c0, a); then c0 -= v -> c1
                        e2 = up.tile([P, 2, N], BF16, tag="e2",
                                     name=f"e2_{b}_{k}_{m}")
                        nc.vector.tensor_sub(e2[:], uvw[:, 0:2, :], uvw[:, 1:3, :])
                        nc.vector.tensor_sub(e2[:, 0, :], e2[:, 0, :], v)
                        if last:
                            f2 = up.tile([P, 2, N], BF16, tag="f2",
                                         name=f"f2_{b}_{m}")
                            nc.vector.tensor_add(f2[:], e2[:], cc[:, k, m, :, :])
                            nc.sync.dma_start(out=oc_d[b, m], in_=f2[:])
                        else:
                            nc.vector.tensor_add(
                                g2_n[:, m, :, :], e2[:], cc[:, k, m, :, :]
                            )
                            nc.vector.tensor_add(
                                gs_n[:, m, :], g2_n[:, m, 0, :], g2_n[:, m, 1, :]
                            )
                    if not last:
                        g2, gs = g2_n, gs_n

            if b + 1 < bpc:
                load_x(b + 1, FP8_X + BF16_X)

    nc.finalize()
    return nc


def _get_nc() -> bass.Bass:
    if "nc" not in _NC_CACHE:
        _NC_CACHE["nc"] = _build_nc()
    return _NC_CACHE["nc"]


def _prep_inputs(x: np.ndarray, coeffs: np.ndarray):
    x = np.ascontiguousarray(x, dtype=np.float32)
    coeffs = np.ascontiguousarray(coeffs, dtype=np.float32)

    xr_f = x[:, 0].reshape(B, KO, P, N)
    xi_f = x[:, 1].reshape(B, KO, P, N)
    xr = xr_f.astype(BF16_NP)
    xi = xi_f.astype(BF16_NP)
    xs = (xr_f + xi_f).astype(BF16_NP)
    # fp8 operands are pre-scaled into e4m3's normal range (X entries are
    # ~N(0, 0.02^2) -- raw they'd be almost entirely subnormal):
    #   X shipped as fp8(64*X), G kept as fp8(16*G).
    xr8 = (xr_f * 64).astype(FP8_NP)
    xi8 = (xi_f * 64).astype(FP8_NP)
    xn8 = (-xi_f * 64).astype(FP8_NP)

    crT = np.ascontiguousarray(coeffs[:, 0].transpose(0, 2, 1))  # [DEG, N, N]
    ciT = np.ascontiguousarray(coeffs[:, 1].transpose(0, 2, 1))
    # cc[k, ko, j, p, n]: j=0 -> Ci^T, j=1 -> Cr^T.  For the non-transition
    # fp8 steps (k > k_trans) the coefficients carry the 16x G-scale.
    k_trans = DEG - 1 - N_FP8_STEPS
    cc_f = np.stack(
        [ciT[: DEG - 1].reshape(DEG - 1, KO, P, N),
         crT[: DEG - 1].reshape(DEG - 1, KO, P, N)], axis=2
    ).copy()
    cc_f[k_trans + 1 :] *= 16.0
    cc = cc_f.astype(BF16_NP)
    cst = (crT[k_trans] + ciT[k_trans]).reshape(KO, P, N).astype(BF16_NP)
    g0c8 = (16.0 * np.stack(
        [ciT[DEG - 1].reshape(KO, P, N), crT[DEG - 1].reshape(KO, P, N)], axis=1
    )).astype(FP8_NP)

    in_maps = []
    for c in range(NCORES):
        sl = slice(c * BPC, (c + 1) * BPC)
        in_maps.append(
            {
                "xr": np.ascontiguousarray(xr[sl]),
                "xi": np.ascontiguousarray(xi[sl]),
                "xs": np.ascontiguousarray(xs[sl]),
                "xr8": np.ascontiguousarray(xr8[sl]),
                "xi8": np.ascontiguousarray(xi8[sl]),
                "xn8": np.ascontiguousarray(xn8[sl]),
                "cc": cc,
                "cst": cst,
                "g0c8": g0c8,
            }
        )
    return in_maps


def _assemble_output(results) -> np.ndarray:
    out = np.empty((B, 2, N, N), dtype=np.float32)
    for c in range(NCORES):
        oc = results[c]["oc"].astype(np.float32)      # [BPC, KO, P, 2, N]
        re = oc[:, :, :, 1, :].reshape(BPC, N, N)     # res^T rows = ko*P+p
        im = oc[:, :, :, 0, :].reshape(BPC, N, N)
        for b in range(BPC):
            out[c * BPC + b, 0] = re[b].T
            out[c * BPC + b, 1] = im[b].T
    return out


def run_sharded(x: np.ndarray, coeffs: np.ndarray, **run_kwargs):
    nc = _get_nc()
    in_maps = _prep_inputs(x, coeffs)
    res = run_bass_kernel_spmd(nc, in_maps, list(range(NCORES)), **run_kwargs)
    return _assemble_output(res.results), res


def kernel(x: np.ndarray, coeffs: np.ndarray) -> np.ndarray:
    out, _ = run_sharded(x, coeffs)
    return out
